# revision 2
# baseline (speedup 1.0000x reference)
"""BMMRemapper Trainium2 kernel (dma_gather version).

Math: out[n,c,q] = sum_k x[n,c,k] * mat[n,q,k] where mat is the bilinear
interpolation matrix built from grid (4 nonzeros per row q: rows lin, lin+1,
lin+48, lin+49 of x^T with weights (1-a)(1-b), (1-a)b, a(1-b), ab).

Instead of a dense 2304x2304 BMM we exploit the 4-sparsity: the host stages
a quad-row table xq[k] = [x^T[k], x^T[k+1], x^T[k+48], x^T[k+49]] in fp16
(pure data movement + dtype cast), so one 1KB-contiguous fetch per output
pixel covers all four corner rows.

Gather engine: the Anthropic SWDGE batch gather (InstDMAGatherAnt via
gpsimd.dma_gather) generates ALL descriptors of a chunk in ONE ucode launch
(~1.0us fixed + 0.34ns/descriptor), unlike indirect_dma_start which pays the
~1.4us launch per 128 descriptors (18 launches = ~25us wall in the previous
version). 4 chunked gathers (3/5/5/5 tiles) pipeline descriptor generation,
the 16-ring SDMA transfer (2.36MB fp16), and the combine.

dma_gather semantics: gathered item i lands at out[i%128, i//128, :], and
indices are read as int16 from a [16, n/16] wrapped layout (idx n at
partition n%16, free n//16), replicated across the 8 GPSIMD core blocks.
We therefore compute the indices directly in that layout from gfold, a
host-staged copy of the grid pre-permuted so partition p holds the grid
values of pixels q === p (mod 16) (replicated 8x; data movement only).

Combine: out[q, c] = c00*A + c01*B + c10*C + c11*D collapses to 3 wide DVE
ops per chunk using a stride-0 (broadcast) coefficient AP:
  mul[p, t, j, c] = gath[p, t, j, c] * cquad[p, t, j]  (c broadcast)
  s1[p, t, i, c]  = mul[p, t, i, c] + mul[p, t, 2+i, c]
  out[p, t, c]    = s1[p, t, 0, c] + s1[p, t, 1, c]
Coefficients (with the all-batch disk mask folded in) are computed once on
DVE in fp32 from gcoef/gall and written as fp16 into cquad[p, 4t+j].

Sharding: batch-parallel, one batch per NeuronCore (N=8 = n_cores), no
cross-core communication. Every core receives the full grid (tiny) and
computes the disk mask locally.

Layouts (q = output pixel, 0..2303; t = q//128; p = q%128):
  xq     (2304, 512) f16 : quad-row table (row k -> 4 corner rows for lin=k).
  gcoef  (128, 36)   f32 : own-batch grid, [p, 2*t+coord]          (coeffs).
  gall   (128, 288)  f32 : all-batch grid, [p, 16*t + 2*m + coord] (mask).
  gfold  (128, 288)  f32 : own-batch grid, [p, 2*s+coord] = g[16s + p%16]
                           (idx chain; replicated across 16-part blocks).
  outp   (128, 2304) f16 : [p, t*128 + c]  (host re-permutes to (c, q)).
"""

import numpy as np

N, H, W, C = 8, 48, 48, 128
HW = H * W            # 2304
NT = HW // 128        # 18
EPS = 1e-5
CLIP_HI = float(np.float32(float(H - 1) - EPS))  # 46.99999 (f32)

# gather chunk boundaries, in 128-pixel tiles
CHUNKS = [(0, 3), (3, 8), (8, 13), (13, 18)]

_CACHE = {}


def _build_nc():
    from contextlib import ExitStack

    import concourse.bacc as bacc
    import concourse.mybir as mybir
    import concourse.tile as tile

    dt = mybir.dt
    f32, f16, i32, i16 = dt.float32, dt.float16, dt.int32, dt.int16
    Alu = mybir.AluOpType

    nc = bacc.Bacc("TRN2", target_bir_lowering=False, debug=False, num_devices=N)

    xq = nc.dram_tensor("xq", [HW, 4 * C], f16, kind="ExternalInput")
    gcoef = nc.dram_tensor("gcoef", [128, 2 * NT], f32, kind="ExternalInput")
    gall = nc.dram_tensor("gall", [128, 16 * NT], f32, kind="ExternalInput")
    gfold = nc.dram_tensor("gfold", [128, 2 * (HW // 16)], f32, kind="ExternalInput")
    outp = nc.dram_tensor("outp", [128, HW], f16, kind="ExternalOutput")

    NS = HW // 16  # 144 idx columns

    with tile.TileContext(nc) as tc, ExitStack() as ctx:
        pool = ctx.enter_context(tc.tile_pool(name="p", bufs=1))
        v = nc.vector
        gp = nc.gpsimd

        # ---- load grid layouts (HWDGE); gfold first (idx critical path) ----
        g_fold = pool.tile([128, 2 * NS], f32)
        g_coef = pool.tile([128, 2 * NT], f32)
        g_all = pool.tile([128, 16 * NT], f32)
        nc.sync.dma_start(g_fold[:], gfold.ap())
        nc.sync.dma_start(g_coef[:], gcoef.ap())
        nc.sync.dma_start(g_all[:], gall.ap())

        # ---- DVE: idx chain in wrapped layout -----------------------------
        # cs = clip(g) - 0.5 (shifted clip bounds make the int cast's
        # round-to-nearest an exact floor for non-integer coords; exactly-
        # integer coords may floor one lower, which yields the identical
        # bilinear result as the weight crosses 0/1).
        csf = pool.tile([128, 2 * NS], f32)
        fif = pool.tile([128, 2 * NS], i32)
        flf = pool.tile([128, 2 * NS], f32)
        linf = pool.tile([128, NS], f32)
        idx16 = pool.tile([128, NS], i16)
        gts = []
        # chunk 0's idx columns first so the first gather launches early
        col_ranges = [(CHUNKS[0][0] * 16, CHUNKS[0][1] * 16),
                      (CHUNKS[0][1] * 16, 2 * NS)]
        with tc.high_priority():
            c0, c1 = col_ranges[0]
            i0, i1 = c0 // 2, c1 // 2
            v.tensor_scalar(csf[:, c0:c1], g_fold[:, c0:c1], EPS, CLIP_HI,
                            Alu.max, Alu.min)
            v.tensor_scalar(csf[:, c0:c1], csf[:, c0:c1], -0.5, None, Alu.add)
            v.tensor_copy(fif[:, c0:c1], csf[:, c0:c1])
            v.tensor_copy(flf[:, c0:c1], fif[:, c0:c1])
            v.scalar_tensor_tensor(
                linf[:, i0:i1], flf[:, c0:c1:2], float(W), flf[:, c0 + 1:c1:2],
                Alu.mult, Alu.add,
            )
            v.tensor_copy(idx16[:, i0:i1], linf[:, i0:i1])
            t0, t1 = CHUNKS[0]
            ntk = t1 - t0
            ni = ntk * 128
            gt = pool.tile([128, ntk * 4 * C], f16, tag="G0")
            gp.dma_gather(
                gt[:].rearrange("p (t e) -> p t e", e=4 * C),
                xq.ap(),
                idx16[:, t0 * 8: t1 * 8],
                ni, ni, 4 * C,
            )
            gts.append(gt)

        # remaining idx columns, then their gather launches
        c0, c1 = col_ranges[1]
        i0, i1 = c0 // 2, c1 // 2
        v.tensor_scalar(csf[:, c0:c1], g_fold[:, c0:c1], EPS, CLIP_HI,
                        Alu.max, Alu.min)
        v.tensor_scalar(csf[:, c0:c1], csf[:, c0:c1], -0.5, None, Alu.add)
        v.tensor_copy(fif[:, c0:c1], csf[:, c0:c1])
        v.tensor_copy(flf[:, c0:c1], fif[:, c0:c1])
        v.scalar_tensor_tensor(
            linf[:, i0:i1], flf[:, c0:c1:2], float(W), flf[:, c0 + 1:c1:2],
            Alu.mult, Alu.add,
        )
        v.tensor_copy(idx16[:, i0:i1], linf[:, i0:i1])
        for k, (t0, t1) in enumerate(CHUNKS[1:], start=1):
            ntk = t1 - t0
            ni = ntk * 128
            gt = pool.tile([128, ntk * 4 * C], f16, tag=f"G{k}")
            gp.dma_gather(
                gt[:].rearrange("p (t e) -> p t e", e=4 * C),
                xq.ap(),
                idx16[:, t0 * 8: t1 * 8],
                ni, ni, 4 * C,
            )
            gts.append(gt)

        # ---- DVE: coefficient chain (q = t*128+p layout) -------------------
        cs = pool.tile([128, 2 * NT], f32)
        v.tensor_scalar(cs[:], g_coef[:], EPS, CLIP_HI, Alu.max, Alu.min)
        v.tensor_scalar(cs[:], cs[:], -0.5, None, Alu.add)
        fi = pool.tile([128, 2 * NT], i32)
        v.tensor_copy(fi[:], cs[:])
        flr = pool.tile([128, 2 * NT], f32)
        v.tensor_copy(flr[:], fi[:])

        # disk mask = AND over batches/coords of in-bounds test
        g_all3 = g_all[:].rearrange("p (t m) -> p t m", m=16)
        mn = pool.tile([128, NT], f32)
        mx = pool.tile([128, NT], f32)
        v.tensor_reduce(mn[:], g_all3, mybir.AxisListType.X, Alu.min)
        v.tensor_reduce(mx[:], g_all3, mybir.AxisListType.X, Alu.max)
        mge = pool.tile([128, NT], f32)
        mle = pool.tile([128, NT], f32)
        v.tensor_scalar(mge[:], mn[:], -0.5, None, Alu.is_ge)
        v.tensor_scalar(mle[:], mx[:], float(H) - 0.5, None, Alu.is_le)
        maskf = pool.tile([128, NT], f32)
        v.tensor_tensor(maskf[:], mge[:], mle[:], Alu.mult)

        # bilinear coefficients; fr = frac - 0.5 (shifted), +0.5 folded in.
        fr = pool.tile([128, 2 * NT], f32)
        v.tensor_tensor(fr[:], cs[:], flr[:], Alu.subtract)
        fra, frb = fr[:, 0::2], fr[:, 1::2]
        fa0 = pool.tile([128, NT], f32)      # 1-a = 0.5 - fra
        fb0 = pool.tile([128, NT], f32)      # 1-b = 0.5 - frb
        v.tensor_scalar(fa0[:], fra, -1.0, 0.5, Alu.mult, Alu.add)
        v.tensor_scalar(fb0[:], frb, -1.0, 0.5, Alu.mult, Alu.add)
        fa0m = pool.tile([128, NT], f32)     # (1-a)*mask
        fa1m = pool.tile([128, NT], f32)     # a*mask = (fra+0.5)*mask
        v.tensor_tensor(fa0m[:], fa0[:], maskf[:], Alu.mult)
        v.scalar_tensor_tensor(fa1m[:], fra, 0.5, maskf[:], Alu.add, Alu.mult)
        # cquad[p, 4t+j] f16: j = 0:c00, 1:c01, 2:c10, 3:c11
        cquad = pool.tile([128, 4 * NT], f16)
        v.tensor_tensor(cquad[:, 0::4], fa0m[:], fb0[:], Alu.mult)
        v.scalar_tensor_tensor(cquad[:, 1::4], frb, 0.5, fa0m[:], Alu.add,
                               Alu.mult)
        v.tensor_tensor(cquad[:, 2::4], fa1m[:], fb0[:], Alu.mult)
        v.scalar_tensor_tensor(cquad[:, 3::4], frb, 0.5, fa1m[:], Alu.add,
                               Alu.mult)

        # ---- combine per chunk: 3 wide DVE ops, coef broadcast over c ------
        for k, (t0, t1) in enumerate(CHUNKS):
            ntk = t1 - t0
            gt4 = gts[k][:].rearrange("p (t j c) -> p t j c", j=4, c=C)
            cb = (cquad[:, 4 * t0: 4 * t1]
                  .rearrange("p (t j) -> p t j", j=4)[:, :, :, None]
                  .broadcast_to((128, ntk, 4, C)))
            mul = pool.tile([128, ntk * 4 * C], f16, tag=f"M{k}")
            mul4 = mul[:].rearrange("p (t j c) -> p t j c", j=4, c=C)
            v.tensor_tensor(mul4, gt4, cb, Alu.mult)
            s1 = pool.tile([128, ntk * 2 * C], f16, tag=f"S{k}")
            s14 = s1[:].rearrange("p (t j c) -> p t j c", j=2, c=C)
            v.tensor_tensor(s14, mul4[:, :, 0:2, :], mul4[:, :, 2:4, :],
                            Alu.add)
            outk = pool.tile([128, ntk * C], f16, tag=f"O{k}")
            out3 = outk[:].rearrange("p (t c) -> p t c", c=C)
            v.tensor_tensor(out3, s14[:, :, 0, :], s14[:, :, 1, :], Alu.add)
            nc.sync.dma_start(outp.ap()[:, t0 * C: t1 * C], outk[:])

    nc.compile()
    return nc


def _get_nc():
    if "nc" not in _CACHE:
        _CACHE["nc"] = _build_nc()
    return _CACHE["nc"]


def _stage_inputs(x, grid):
    """Build the per-core input maps (data movement / dtype cast only)."""
    x = np.ascontiguousarray(x, dtype=np.float32)
    grid = np.ascontiguousarray(grid, dtype=np.float32)
    xr = x.reshape(N, C, HW)
    gr = grid.reshape(N, HW, 2)

    # quad-row table: xq[n][k] = [xT[k], xT[k+1], xT[k+48], xT[k+49]]  (fp16)
    xt = np.zeros((N, HW + W + 2, C), dtype=np.float16)
    xt[:, :HW] = xr.transpose(0, 2, 1)
    xq = np.empty((N, HW, 4 * C), dtype=np.float16)
    xq[:, :, 0 * C: 1 * C] = xt[:, 0: HW]
    xq[:, :, 1 * C: 2 * C] = xt[:, 1: HW + 1]
    xq[:, :, 2 * C: 3 * C] = xt[:, W: HW + W]
    xq[:, :, 3 * C: 4 * C] = xt[:, W + 1: HW + W + 1]

    # gcoef[n][p, 2t+c] = gr[n, t*128+p, c]
    gc = gr.reshape(N, NT, 128, 2).transpose(0, 2, 1, 3)  # [n, p, t, c]
    gcoef = np.ascontiguousarray(gc.reshape(N, 128, 2 * NT))

    # gall[p, 16t+2m+c] = gr[m, t*128+p, c]   (same for all cores)
    ga = gr.reshape(N, NT, 128, 2).transpose(2, 1, 0, 3)  # [p, t, m, c]
    gall = np.ascontiguousarray(ga.reshape(128, 16 * NT))

    # gfold[n][p, 2s+c] = gr[n, 16s + p%16, c], replicated over the 8
    # 16-partition blocks (dma_gather's wrapped idx layout).
    NS = HW // 16
    gw = gr.reshape(N, NS, 16, 2).transpose(0, 2, 1, 3)   # [n, pp, s, c]
    gfold = np.tile(gw, (1, 8, 1, 1)).reshape(N, 128, 2 * NS)
    gfold = np.ascontiguousarray(gfold)

    return [
        {"xq": xq[n], "gcoef": gcoef[n], "gall": gall, "gfold": gfold[n]}
        for n in range(N)
    ]


def _unstage_output(results):
    """results[n]["outp"] is (128, 2304) f16 = [p, t*128+c] -> (N, C, H, W)."""
    out = np.empty((N, C, H, W), dtype=np.float32)
    for n in range(N):
        o = results[n]["outp"].astype(np.float32).reshape(128, NT, C)
        out[n] = o.transpose(2, 1, 0).reshape(C, H, W)   # [c, q=t*128+p]
    return out


def kernel(x, grid):
    from concourse import bass_utils

    nc = _get_nc()
    in_maps = _stage_inputs(x, grid)
    res = bass_utils.run_bass_kernel_spmd(nc, in_maps, core_ids=list(range(N)))
    return _unstage_output(res.results)


# revision 5
# speedup vs baseline: 1.3204x; 1.3204x over previous
"""BMMRemapper Trainium2 kernel.

Math: out[n,c,q] = sum_k x[n,c,k] * mat[n,q,k] where mat is the bilinear
interpolation matrix built from grid (4 nonzeros per row q: rows lin, lin+1,
lin+48, lin+49 of x^T with weights (1-a)(1-b), (1-a)b, a(1-b), ab).

Instead of a dense 2304x2304 BMM we exploit the 4-sparsity: the host stages
a quad-row table xq[k] = [x^T[k], x^T[k+1], x^T[k+48], x^T[k+49]] in fp16
(pure data movement + dtype cast), so ONE indirect-DMA descriptor per output
pixel fetches all four corner rows (1 KB contiguous). The HW vector-indirect
DMA consumes exactly one offset per partition per instruction, so the gather
is 18 instructions of 128 descriptors each; their ~1.4us SWDGE ucode
launches serialize on GPSIMD and set this kernel's floor (~25 us). (The
batched InstDMAGatherAnt ucode was measured at ~9ns/descriptor -- its int16
scalar unpack loop erases the batching win, so per-128 indirect launches at
994ns fixed + 0.74ns/desc remain optimal.)

Head: a tiny separate gcoef2 input (tiles 0-1 grid columns) is the kernel's
first DMA; its 4-op int chain puts idx for two tiles on GPSIMD ~1.3us before
the full gcoef path would.

Combine: out = c00*A + c01*B + c10*C + c11*D collapses to 3 wide DVE ops per
tile-group using a stride-0 (broadcast) coefficient AP (HW-verified):
  mul[p, t, j, c] = gath[p, t, j, c] * cquad[p, t, j]  (c broadcast)
  s1[p, t, i, c]  = mul[p, t, i, c] + mul[p, t, 2+i, c]
  out[p, t, c]    = s1[p, t, 0, c] + s1[p, t, 1, c]
Gathers write slices of group-sized tiles so the wide ops span contiguous
APs. All-DVE (~0.5us per tile incl. overheads) hides under the launch train;
the last tiles run as single-tile groups to minimize the post-train tail.

Sharding: batch-parallel, one batch per NeuronCore (N=8 = n_cores), no
cross-core communication. The disk mask couples batches (all-batch AND), so
every core receives the full grid (tiny) and computes the mask locally.

Layouts (q = output pixel, 0..2303; t = q//128; p = q%128):
  xq     (2304, 512) f16 : quad-row table (row k -> 4 corner rows for lin=k).
  gcoef2 (128, 4)    f32 : tiles 0-1 grid cols only (earliest idx path).
  gcoef  (128, 36)   f32 : own-batch grid, [p, 2*t+coord].
  gall   (128, 288)  f32 : all-batch grid, [p, 16*t + 2*m + coord].
  outp   (128, 2304) f16 : [p, t*128 + c]  (host re-permutes to (c, q)).
"""

import numpy as np

N, H, W, C = 8, 48, 48, 128
HW = H * W            # 2304
NT = HW // 128        # 18
EPS = 1e-5
CLIP_HI = float(np.float32(float(H - 1) - EPS))  # 46.99999 (f32)

# combine/store groups (in tiles): 3-tile groups early, singles at the tail
GROUPS = [(0, 3), (3, 6), (6, 9), (9, 12), (12, 15), (15, 16), (16, 17),
          (17, 18)]

_CACHE = {}


def _build_nc():
    from contextlib import ExitStack

    import concourse.bacc as bacc
    import concourse.bass as bass
    import concourse.mybir as mybir
    import concourse.tile as tile

    dt = mybir.dt
    f32, f16, i32 = dt.float32, dt.float16, dt.int32
    Alu = mybir.AluOpType

    nc = bacc.Bacc("TRN2", target_bir_lowering=False, debug=False, num_devices=N)

    xq = nc.dram_tensor("xq", [HW, 4 * C], f16, kind="ExternalInput")
    gcoef2 = nc.dram_tensor("gcoef2", [128, 4], f32, kind="ExternalInput")
    gcoef = nc.dram_tensor("gcoef", [128, 2 * NT], f32, kind="ExternalInput")
    gall = nc.dram_tensor("gall", [128, 16 * NT], f32, kind="ExternalInput")
    outp = nc.dram_tensor("outp", [128, HW], f16, kind="ExternalOutput")

    # map tile index -> (group index, offset within group)
    t2g = {}
    for k, (t0, t1) in enumerate(GROUPS):
        for tt in range(t0, t1):
            t2g[tt] = (k, tt - t0)

    with tile.TileContext(nc) as tc, ExitStack() as ctx:
        pool = ctx.enter_context(tc.tile_pool(name="p", bufs=1))
        v = nc.vector
        gp = nc.gpsimd

        # group-sized gather destination tiles
        ggs = []
        for k, (t0, t1) in enumerate(GROUPS):
            gg_k = pool.tile([128, (t1 - t0) * 4 * C], f16, tag=f"GG{k}",
                             name=f"gg{k}")
            ggs.append(gg_k)

        def gather_tile(t, idx_ap):
            k, off = t2g[t]
            gp.indirect_dma_start(
                out=ggs[k][:, off * 4 * C: (off + 1) * 4 * C],
                out_offset=None,
                in_=xq.ap(),
                in_offset=bass.IndirectOffsetOnAxis(ap=idx_ap, axis=0),
            )

        # ---- load grid layouts (HWDGE); gcoef2 first (idx critical path) ---
        g_c2 = pool.tile([128, 4], f32)
        g_coef = pool.tile([128, 2 * NT], f32)
        g_all = pool.tile([128, 16 * NT], f32)
        nc.sync.dma_start(g_c2[:], gcoef2.ap())
        nc.sync.dma_start(g_coef[:], gcoef.ap())
        nc.sync.dma_start(g_all[:], gall.ap())

        # ---- DVE: idx chain. cs = clip(g)-0.5 in 2 TS ops; int-cast rounds
        # to nearest == floor for non-integer coords (exact-integer coords may
        # floor one lower: identical bilinear result as the weight hits 0/1).
        # Tiles 0-1 first from the tiny gcoef2 (own DMA), launching their
        # gathers ~1.3us before the full-gcoef path could.
        cs0 = pool.tile([128, 4], f32)
        fi0 = pool.tile([128, 4], i32)
        idx0 = pool.tile([128, 2], i32)
        with tc.high_priority():
            v.tensor_scalar(cs0[:], g_c2[:], EPS, CLIP_HI, Alu.max, Alu.min)
            v.tensor_scalar(cs0[:], cs0[:], -0.5, None, Alu.add)
            v.tensor_copy(fi0[:], cs0[:])
            v.scalar_tensor_tensor(
                idx0[:], fi0[:, 0::2], W, fi0[:, 1::2], Alu.mult, Alu.add
            )
            gather_tile(0, idx0[:, 0:1])
            gather_tile(1, idx0[:, 1:2])

        # remaining 16 tiles' idx from the full gcoef
        cs = pool.tile([128, 2 * NT], f32)
        fi = pool.tile([128, 2 * NT], i32)
        idx = pool.tile([128, NT], i32)
        v.tensor_scalar(cs[:, 4:], g_coef[:, 4:], EPS, CLIP_HI, Alu.max,
                        Alu.min)
        v.tensor_scalar(cs[:, 4:], cs[:, 4:], -0.5, None, Alu.add)
        v.tensor_copy(fi[:, 4:], cs[:, 4:])
        v.scalar_tensor_tensor(
            idx[:, 2:], fi[:, 4::2], W, fi[:, 5::2], Alu.mult, Alu.add
        )
        for t in range(2, NT):
            gather_tile(t, idx[:, t: t + 1])
        # fold tiles 0-1 working values into the full tiles for coefficients
        v.tensor_copy(cs[:, 0:4], cs0[:])
        v.tensor_copy(fi[:, 0:4], fi0[:])

        # ---- DVE: disk mask = AND over batches/coords of in-bounds test ----
        g_all3 = g_all[:].rearrange("p (t m) -> p t m", m=16)
        mn = pool.tile([128, NT], f32)
        mx = pool.tile([128, NT], f32)
        v.tensor_reduce(mn[:], g_all3, mybir.AxisListType.X, Alu.min)
        v.tensor_reduce(mx[:], g_all3, mybir.AxisListType.X, Alu.max)
        mge = pool.tile([128, NT], f32)
        mle = pool.tile([128, NT], f32)
        v.tensor_scalar(mge[:], mn[:], -0.5, None, Alu.is_ge)
        v.tensor_scalar(mle[:], mx[:], float(H) - 0.5, None, Alu.is_le)
        maskf = pool.tile([128, NT], f32)
        v.tensor_tensor(maskf[:], mge[:], mle[:], Alu.mult)

        # ---- DVE: bilinear coefficients -> cquad[p, 4t+j] f16 --------------
        # fr = frac - 0.5 (shifted); the +0.5 folds into the STT/TS ops.
        flr = pool.tile([128, 2 * NT], f32)
        v.tensor_copy(flr[:], fi[:])
        fr = pool.tile([128, 2 * NT], f32)
        v.tensor_tensor(fr[:], cs[:], flr[:], Alu.subtract)
        fra, frb = fr[:, 0::2], fr[:, 1::2]
        fa0 = pool.tile([128, NT], f32)      # 1-a = 0.5 - fra
        fb0 = pool.tile([128, NT], f32)      # 1-b = 0.5 - frb
        v.tensor_scalar(fa0[:], fra, -1.0, 0.5, Alu.mult, Alu.add)
        v.tensor_scalar(fb0[:], frb, -1.0, 0.5, Alu.mult, Alu.add)
        fa0m = pool.tile([128, NT], f32)     # (1-a)*mask
        fa1m = pool.tile([128, NT], f32)     # a*mask = (fra+0.5)*mask
        v.tensor_tensor(fa0m[:], fa0[:], maskf[:], Alu.mult)
        v.scalar_tensor_tensor(fa1m[:], fra, 0.5, maskf[:], Alu.add, Alu.mult)
        cquad = pool.tile([128, 4 * NT], f16)
        v.tensor_tensor(cquad[:, 0::4], fa0m[:], fb0[:], Alu.mult)
        v.scalar_tensor_tensor(cquad[:, 1::4], frb, 0.5, fa0m[:], Alu.add,
                               Alu.mult)
        v.tensor_tensor(cquad[:, 2::4], fa1m[:], fb0[:], Alu.mult)
        v.scalar_tensor_tensor(cquad[:, 3::4], frb, 0.5, fa1m[:], Alu.add,
                               Alu.mult)

        # ---- combine per group: 3 wide DVE ops, coef broadcast over c ------
        for k, (t0, t1) in enumerate(GROUPS):
            ntk = t1 - t0
            gt4 = ggs[k][:].rearrange("p (t j c) -> p t j c", j=4, c=C)
            cb = (cquad[:, 4 * t0: 4 * t1]
                  .rearrange("p (t j) -> p t j", j=4)[:, :, :, None]
                  .broadcast_to((128, ntk, 4, C)))
            mul = pool.tile([128, ntk * 4 * C], f16, tag=f"M{k}")
            mul4 = mul[:].rearrange("p (t j c) -> p t j c", j=4, c=C)
            v.tensor_tensor(mul4, gt4, cb, Alu.mult)
            s1 = pool.tile([128, ntk * 2 * C], f16, tag=f"S{k}")
            s14 = s1[:].rearrange("p (t j c) -> p t j c", j=2, c=C)
            v.tensor_tensor(s14, mul4[:, :, 0:2, :], mul4[:, :, 2:4, :],
                            Alu.add)
            outk = pool.tile([128, ntk * C], f16, tag=f"O{k}")
            out3 = outk[:].rearrange("p (t c) -> p t c", c=C)
            v.tensor_tensor(out3, s14[:, :, 0, :], s14[:, :, 1, :], Alu.add)
            nc.sync.dma_start(outp.ap()[:, t0 * C: t1 * C], outk[:])

    nc.compile()
    return nc


def _get_nc():
    if "nc" not in _CACHE:
        _CACHE["nc"] = _build_nc()
    return _CACHE["nc"]


def _stage_inputs(x, grid):
    """Build the per-core input maps (data movement / dtype cast only)."""
    x = np.ascontiguousarray(x, dtype=np.float32)
    grid = np.ascontiguousarray(grid, dtype=np.float32)
    xr = x.reshape(N, C, HW)
    gr = grid.reshape(N, HW, 2)

    # quad-row table: xq[n][k] = [xT[k], xT[k+1], xT[k+48], xT[k+49]]  (fp16)
    xt = np.zeros((N, HW + W + 2, C), dtype=np.float16)
    xt[:, :HW] = xr.transpose(0, 2, 1)
    xq = np.empty((N, HW, 4 * C), dtype=np.float16)
    xq[:, :, 0 * C: 1 * C] = xt[:, 0: HW]
    xq[:, :, 1 * C: 2 * C] = xt[:, 1: HW + 1]
    xq[:, :, 2 * C: 3 * C] = xt[:, W: HW + W]
    xq[:, :, 3 * C: 4 * C] = xt[:, W + 1: HW + W + 1]

    # gcoef[n][p, 2t+c] = gr[n, t*128+p, c]
    gc = gr.reshape(N, NT, 128, 2).transpose(0, 2, 1, 3)  # [n, p, t, c]
    gcoef = np.ascontiguousarray(gc.reshape(N, 128, 2 * NT))
    gcoef2 = np.ascontiguousarray(gcoef[:, :, 0:4])

    # gall[p, 16t+2m+c] = gr[m, t*128+p, c]   (same for all cores)
    ga = gr.reshape(N, NT, 128, 2).transpose(2, 1, 0, 3)  # [p, t, m, c]
    gall = np.ascontiguousarray(ga.reshape(128, 16 * NT))

    return [
        {"xq": xq[n], "gcoef2": gcoef2[n], "gcoef": gcoef[n], "gall": gall}
        for n in range(N)
    ]


def _unstage_output(results):
    """results[n]["outp"] is (128, 2304) f16 = [p, t*128+c] -> (N, C, H, W)."""
    out = np.empty((N, C, H, W), dtype=np.float32)
    for n in range(N):
        o = results[n]["outp"].astype(np.float32).reshape(128, NT, C)
        out[n] = o.transpose(2, 1, 0).reshape(C, H, W)   # [c, q=t*128+p]
    return out


def kernel(x, grid):
    from concourse import bass_utils

    nc = _get_nc()
    in_maps = _stage_inputs(x, grid)
    res = bass_utils.run_bass_kernel_spmd(nc, in_maps, core_ids=list(range(N)))
    return _unstage_output(res.results)


# revision 7
# speedup vs baseline: 1.3423x; 1.0166x over previous
"""BMMRemapper Trainium2 kernel.

Math: out[n,c,q] = sum_k x[n,c,k] * mat[n,q,k] where mat is the bilinear
interpolation matrix built from grid (4 nonzeros per row q: rows lin, lin+1,
lin+48, lin+49 of x^T with weights (1-a)(1-b), (1-a)b, a(1-b), ab).

Instead of a dense 2304x2304 BMM we exploit the 4-sparsity: the host stages
a quad-row table xq[k] = [x^T[k], x^T[k+1], x^T[k+48], x^T[k+49]] in fp16
(pure data movement + dtype cast), so ONE indirect-DMA descriptor per output
pixel fetches all four corner rows (1 KB contiguous). The HW vector-indirect
DMA consumes exactly one offset per partition per instruction, so the gather
is 18 instructions of 128 descriptors each; their ~1.4us SWDGE ucode
launches (994ns fixed + 0.74ns/descriptor + ~310ns sequencer) serialize on
GPSIMD and set this kernel's floor (~25us train). The batched
InstDMAGatherAnt ucode was measured at ~9ns/descriptor (scalar int16 unpack
loop), erasing its batching advantage, so the per-128 indirect launches are
optimal. Big concurrent DVE ops measurably slow the ucode (SBUF port
contention), so the in-train combine keeps the baseline's small-op ACT+DVE
split.

Head (-1.1us): a tiny separate gcoef2 input (tiles 0-1 grid columns) is the
kernel's first DMA, and the idx chain is 4 ops (clip, shift, int cast, int
STT) instead of 6 (the linearization runs in int32 directly).

Tail (-1.5us): tiles 15-17 combine as single-tile 3-op DVE mega-combines
with a stride-0 (broadcast) coefficient AP (HW-verified) and store
individually (32KB), so the post-train critical path is data-wait + 0.9us
combine + small store instead of ACT chains + a 3-tile store.

Sharding: batch-parallel, one batch per NeuronCore (N=8 = n_cores), no
cross-core communication. The disk mask couples batches (all-batch AND), so
every core receives the full grid (tiny) and computes the mask locally.

Layouts (q = output pixel, 0..2303; t = q//128; p = q%128):
  xq     (2304, 512) f16 : quad-row table (row k -> 4 corner rows for lin=k).
  gcoef2 (128, 4)    f32 : tiles 0-1 grid cols only (earliest idx path).
  gcoef  (128, 36)   f32 : own-batch grid, [p, 2*t+coord].
  gall   (128, 288)  f32 : all-batch grid, [p, 16*t + 2*m + coord].
  outp   (128, 2304) f16 : [p, t*128 + c]  (host re-permutes to (c, q)).
"""

import numpy as np

N, H, W, C = 8, 48, 48, 128
HW = H * W            # 2304
NT = HW // 128        # 18
EPS = 1e-5
CLIP_HI = float(np.float32(float(H - 1) - EPS))  # 46.99999 (f32)

NBULK = 15            # tiles 0..14: baseline-style combine, 3-tile stores
_CACHE = {}


def _build_nc():
    from contextlib import ExitStack

    import concourse.bacc as bacc
    import concourse.bass as bass
    import concourse.mybir as mybir
    import concourse.tile as tile

    dt = mybir.dt
    f32, f16, i32 = dt.float32, dt.float16, dt.int32
    Alu = mybir.AluOpType

    nc = bacc.Bacc("TRN2", target_bir_lowering=False, debug=False, num_devices=N)

    xq = nc.dram_tensor("xq", [HW, 4 * C], f16, kind="ExternalInput")
    gcoef2 = nc.dram_tensor("gcoef2", [128, 4], f32, kind="ExternalInput")
    gcoef = nc.dram_tensor("gcoef", [128, 2 * NT], f32, kind="ExternalInput")
    gall = nc.dram_tensor("gall", [128, 16 * NT], f32, kind="ExternalInput")
    outp = nc.dram_tensor("outp", [128, HW], f16, kind="ExternalOutput")

    with tile.TileContext(nc) as tc, ExitStack() as ctx:
        pool = ctx.enter_context(tc.tile_pool(name="p", bufs=1))
        v = nc.vector
        gp = nc.gpsimd

        # ---- load grid layouts (HWDGE); gcoef2 first (idx critical path) ---
        g_c2 = pool.tile([128, 4], f32)
        g_coef = pool.tile([128, 2 * NT], f32)
        g_all = pool.tile([128, 16 * NT], f32)
        nc.sync.dma_start(g_c2[:], gcoef2.ap())
        nc.sync.dma_start(g_coef[:], gcoef.ap())
        nc.sync.dma_start(g_all[:], gall.ap())

        # ---- DVE idx chain: cs = clip(g)-0.5 (2 TS), int cast (round-to-
        # nearest == floor for non-integer coords; exactly-integer coords may
        # floor one lower, giving the identical bilinear result as the weight
        # crosses 0/1), then lin = r*48+s in int32. Tiles 0-1 first from the
        # tiny gcoef2 (its own first-in-kernel DMA).
        cs0 = pool.tile([128, 4], f32)
        fi0 = pool.tile([128, 4], i32)
        idx0 = pool.tile([128, 2], i32)
        gts = [None] * NT

        def gather(t, idx_ap, gt):
            gp.indirect_dma_start(
                out=gt[:],
                out_offset=None,
                in_=xq.ap(),
                in_offset=bass.IndirectOffsetOnAxis(ap=idx_ap, axis=0),
            )

        with tc.high_priority():
            v.tensor_scalar(cs0[:], g_c2[:], EPS, CLIP_HI, Alu.max, Alu.min)
            v.tensor_scalar(cs0[:], cs0[:], -0.5, None, Alu.add)
            v.tensor_copy(fi0[:], cs0[:])
            v.scalar_tensor_tensor(
                idx0[:], fi0[:, 0::2], W, fi0[:, 1::2], Alu.mult, Alu.add
            )
            for t in (0, 1):
                gt_t = pool.tile([128, 4 * C], f16, tag=f"G{t}", name=f"g{t}")
                gather(t, idx0[:, t: t + 1], gt_t)
                gts[t] = gt_t

        # remaining 16 tiles' idx from the full gcoef
        cs = pool.tile([128, 2 * NT], f32)
        fi = pool.tile([128, 2 * NT], i32)
        idx = pool.tile([128, NT], i32)
        v.tensor_scalar(cs[:, 4:], g_coef[:, 4:], EPS, CLIP_HI, Alu.max,
                        Alu.min)
        v.tensor_scalar(cs[:, 4:], cs[:, 4:], -0.5, None, Alu.add)
        v.tensor_copy(fi[:, 4:], cs[:, 4:])
        v.scalar_tensor_tensor(
            idx[:, 2:], fi[:, 4::2], W, fi[:, 5::2], Alu.mult, Alu.add
        )
        for t in range(2, NT):
            gt_t = pool.tile([128, 4 * C], f16, tag=f"G{t}", name=f"g{t}")
            gather(t, idx[:, t: t + 1], gt_t)
            gts[t] = gt_t
        # fold tiles 0-1 working values into the full tiles for coefficients
        v.tensor_copy(cs[:, 0:4], cs0[:])
        v.tensor_copy(fi[:, 0:4], fi0[:])

        # ---- DVE: disk mask = AND over batches/coords of in-bounds test ----
        g_all3 = g_all[:].rearrange("p (t m) -> p t m", m=16)
        mn = pool.tile([128, NT], f32)
        mx = pool.tile([128, NT], f32)
        v.tensor_reduce(mn[:], g_all3, mybir.AxisListType.X, Alu.min)
        v.tensor_reduce(mx[:], g_all3, mybir.AxisListType.X, Alu.max)
        mge = pool.tile([128, NT], f32)
        mle = pool.tile([128, NT], f32)
        v.tensor_scalar(mge[:], mn[:], -0.5, None, Alu.is_ge)
        v.tensor_scalar(mle[:], mx[:], float(H) - 0.5, None, Alu.is_le)
        maskf = pool.tile([128, NT], f32)
        v.tensor_tensor(maskf[:], mge[:], mle[:], Alu.mult)

        # ---- DVE: bilinear coefficients (f32) + fp16 cquad for the tail ----
        # fr = frac - 0.5 (shifted); the +0.5 folds into the STT/TS ops.
        flr = pool.tile([128, 2 * NT], f32)
        v.tensor_copy(flr[:], fi[:])
        fr = pool.tile([128, 2 * NT], f32)
        v.tensor_tensor(fr[:], cs[:], flr[:], Alu.subtract)
        fra, frb = fr[:, 0::2], fr[:, 1::2]
        fa0 = pool.tile([128, NT], f32)      # 1-a = 0.5 - fra
        fb0 = pool.tile([128, NT], f32)      # 1-b = 0.5 - frb
        v.tensor_scalar(fa0[:], fra, -1.0, 0.5, Alu.mult, Alu.add)
        v.tensor_scalar(fb0[:], frb, -1.0, 0.5, Alu.mult, Alu.add)
        fa0m = pool.tile([128, NT], f32)     # (1-a)*mask
        fa1m = pool.tile([128, NT], f32)     # a*mask = (fra+0.5)*mask
        v.tensor_tensor(fa0m[:], fa0[:], maskf[:], Alu.mult)
        v.scalar_tensor_tensor(fa1m[:], fra, 0.5, maskf[:], Alu.add, Alu.mult)
        c00 = pool.tile([128, NT], f32)
        c01 = pool.tile([128, NT], f32)
        c10 = pool.tile([128, NT], f32)
        c11 = pool.tile([128, NT], f32)
        v.tensor_tensor(c00[:], fa0m[:], fb0[:], Alu.mult)
        v.scalar_tensor_tensor(c01[:], frb, 0.5, fa0m[:], Alu.add, Alu.mult)
        v.tensor_tensor(c10[:], fa1m[:], fb0[:], Alu.mult)
        v.scalar_tensor_tensor(c11[:], frb, 0.5, fa1m[:], Alu.add, Alu.mult)
        # fp16 interleaved coefficients for the tail tiles' mega-combine
        cquad = pool.tile([128, 4 * NT], f16)
        v.tensor_copy(cquad[:, 0::4], c00[:])
        v.tensor_copy(cquad[:, 1::4], c01[:])
        v.tensor_copy(cquad[:, 2::4], c10[:])
        v.tensor_copy(cquad[:, 3::4], c11[:])

        # ---- bulk combine (tiles 0..14): baseline ACT+DVE small-op split ---
        # ACT does two products (per-partition activation scale); DVE does
        # two fused multiply-adds + the final add. Small ops keep SBUF port
        # pressure off the concurrently-running SWDGE ucode.
        for k in range(NBULK // 3):
            out_k = pool.tile([128, 3 * C], f16, tag=f"O{k}", name=f"o{k}")
            for tt in range(3):
                t = k * 3 + tt
                g = gts[t]
                A = g[:, 0 * C: 1 * C]
                B = g[:, 1 * C: 2 * C]
                Cr = g[:, 2 * C: 3 * C]
                D = g[:, 3 * C: 4 * C]
                u0 = pool.tile([128, C], f16, tag=f"u0_{t}", name=f"u0_{t}")
                u1 = pool.tile([128, C], f16, tag=f"u1_{t}", name=f"u1_{t}")
                v0 = pool.tile([128, C], f16, tag=f"v0_{t}", name=f"v0_{t}")
                v1 = pool.tile([128, C], f16, tag=f"v1_{t}", name=f"v1_{t}")
                if t < 4:
                    # first tiles' products on DVE (idle at early data
                    # arrival): trims ACT's queue so a late ACT start can
                    # never backlog into the final tiles
                    v.tensor_scalar(u0[:], A, c00[:, t: t + 1], None,
                                    Alu.mult)
                    v.tensor_scalar(u1[:], B, c01[:, t: t + 1], None,
                                    Alu.mult)
                else:
                    nc.scalar.activation(
                        u0[:], A, mybir.ActivationFunctionType.Copy,
                        scale=c00[:, t: t + 1],
                    )
                    nc.scalar.activation(
                        u1[:], B, mybir.ActivationFunctionType.Copy,
                        scale=c01[:, t: t + 1],
                    )
                v.scalar_tensor_tensor(
                    v0[:], Cr, c10[:, t: t + 1], u0[:], Alu.mult, Alu.add
                )
                v.scalar_tensor_tensor(
                    v1[:], D, c11[:, t: t + 1], u1[:], Alu.mult, Alu.add
                )
                v.tensor_tensor(
                    out_k[:, tt * C: (tt + 1) * C], v0[:], v1[:], Alu.add
                )
            nc.sync.dma_start(
                outp.ap()[:, k * 3 * C: (k + 1) * 3 * C], out_k[:]
            )

        # ---- tail (tiles 15..17): 3-op mega-combine + single-tile stores ---
        for t in range(NBULK, NT):
            gt4 = (gts[t][:].rearrange("p (j c) -> p j c", j=4, c=C)
                   [:, None, :, :])
            cb = (cquad[:, 4 * t: 4 * (t + 1)]
                  .rearrange("p (t j) -> p t j", j=4)[:, :, :, None]
                  .broadcast_to((128, 1, 4, C)))
            mul = pool.tile([128, 4 * C], f16, tag=f"M{t}", name=f"m{t}")
            mul4 = mul[:].rearrange("p (t j c) -> p t j c", j=4, c=C)
            v.tensor_tensor(mul4, gt4, cb, Alu.mult)
            s1 = pool.tile([128, 2 * C], f16, tag=f"S{t}", name=f"s{t}")
            s14 = s1[:].rearrange("p (t j c) -> p t j c", j=2, c=C)
            v.tensor_tensor(s14, mul4[:, :, 0:2, :], mul4[:, :, 2:4, :],
                            Alu.add)
            out_t = pool.tile([128, C], f16, tag=f"O_{t}", name=f"ot{t}")
            out3 = out_t[:].rearrange("p (t c) -> p t c", c=C)
            v.tensor_tensor(out3, s14[:, :, 0, :], s14[:, :, 1, :], Alu.add)
            nc.sync.dma_start(outp.ap()[:, t * C: (t + 1) * C], out_t[:])

    nc.compile()
    return nc


def _get_nc():
    if "nc" not in _CACHE:
        _CACHE["nc"] = _build_nc()
    return _CACHE["nc"]


def _stage_inputs(x, grid):
    """Build the per-core input maps (data movement / dtype cast only)."""
    x = np.ascontiguousarray(x, dtype=np.float32)
    grid = np.ascontiguousarray(grid, dtype=np.float32)
    xr = x.reshape(N, C, HW)
    gr = grid.reshape(N, HW, 2)

    # quad-row table: xq[n][k] = [xT[k], xT[k+1], xT[k+48], xT[k+49]]  (fp16)
    xt = np.zeros((N, HW + W + 2, C), dtype=np.float16)
    xt[:, :HW] = xr.transpose(0, 2, 1)
    xq = np.empty((N, HW, 4 * C), dtype=np.float16)
    xq[:, :, 0 * C: 1 * C] = xt[:, 0: HW]
    xq[:, :, 1 * C: 2 * C] = xt[:, 1: HW + 1]
    xq[:, :, 2 * C: 3 * C] = xt[:, W: HW + W]
    xq[:, :, 3 * C: 4 * C] = xt[:, W + 1: HW + W + 1]

    # gcoef[n][p, 2t+c] = gr[n, t*128+p, c]
    gc = gr.reshape(N, NT, 128, 2).transpose(0, 2, 1, 3)  # [n, p, t, c]
    gcoef = np.ascontiguousarray(gc.reshape(N, 128, 2 * NT))
    gcoef2 = np.ascontiguousarray(gcoef[:, :, 0:4])

    # gall[p, 16t+2m+c] = gr[m, t*128+p, c]   (same for all cores)
    ga = gr.reshape(N, NT, 128, 2).transpose(2, 1, 0, 3)  # [p, t, m, c]
    gall = np.ascontiguousarray(ga.reshape(128, 16 * NT))

    return [
        {"xq": xq[n], "gcoef2": gcoef2[n], "gcoef": gcoef[n], "gall": gall}
        for n in range(N)
    ]


def _unstage_output(results):
    """results[n]["outp"] is (128, 2304) f16 = [p, t*128+c] -> (N, C, H, W)."""
    out = np.empty((N, C, H, W), dtype=np.float32)
    for n in range(N):
        o = results[n]["outp"].astype(np.float32).reshape(128, NT, C)
        out[n] = o.transpose(2, 1, 0).reshape(C, H, W)   # [c, q=t*128+p]
    return out


def kernel(x, grid):
    from concourse import bass_utils

    nc = _get_nc()
    in_maps = _stage_inputs(x, grid)
    res = bass_utils.run_bass_kernel_spmd(nc, in_maps, core_ids=list(range(N)))
    return _unstage_output(res.results)


# revision 8
# speedup vs baseline: 1.3834x; 1.0306x over previous
"""BMMRemapper Trainium2 kernel.

Math: out[n,c,q] = sum_k x[n,c,k] * mat[n,q,k] where mat is the bilinear
interpolation matrix built from grid (4 nonzeros per row q: rows lin, lin+1,
lin+48, lin+49 of x^T with weights (1-a)(1-b), (1-a)b, a(1-b), ab).

Instead of a dense 2304x2304 BMM we exploit the 4-sparsity: the host stages
a quad-row table xq[k] = [x^T[k], x^T[k+1], x^T[k+48], x^T[k+49]] in fp16
(pure data movement + dtype cast), so ONE indirect-DMA descriptor per output
pixel fetches all four corner rows (1 KB contiguous). The HW vector-indirect
DMA consumes exactly one offset per partition per instruction, so the gather
is 18 instructions of 128 descriptors each; their ~1.4us SWDGE ucode
launches (994ns fixed + 0.74ns/desc + ~310ns sequencer) serialize on GPSIMD
AND pace the SDMA transfer stream (descriptors arrive at 128KB/1.4us; the
16 rings drain each launch in ~1.6us), so first-launch time sets the whole
pipeline's phase. Alternatives measured and rejected: InstDMAGatherAnt
batched ucode runs at ~9ns/descriptor (scalar int16 unpack), erasing its
batching advantage; ap_gather's Pool datapath (~150GB/s) plus its forced
channel-on-partition layout has no viable combine engine.

Head (-1.2us vs the single-gcoef variant): a tiny separate gcoef2 input
(tiles 0-1 grid columns) is the kernel's first DMA, and the idx chain is 4
ops (clip+shift in 2 TS, int32 cast, int32 STT linearization).

The fp16 table halves the gathered bytes so the SDMA transfers, the combine
(ACT: two products via per-partition activation scale; DVE: two fused
multiply-adds + one add per tile) and the chunked output stores all hide
under the launch-train wall.

Sharding: batch-parallel, one batch per NeuronCore (N=8 = n_cores), no
cross-core communication. The disk mask couples batches (all-batch AND), so
every core receives the full grid (tiny) and computes the mask locally.

Layouts (q = output pixel, 0..2303; t = q//128; p = q%128):
  xq     (2304, 512) f16 : quad-row table (row k -> 4 corner rows for lin=k).
  gcoef2 (128, 4)    f32 : tiles 0-1 grid cols only (earliest idx path).
  gcoef  (128, 36)   f32 : own-batch grid, [p, 2*t+coord].
  gall   (128, 288)  f32 : all-batch grid, [p, 16*t + 2*m + coord].
  outp   (128, 2304) f16 : [p, t*128 + c]  (host re-permutes to (c, q)).
"""

import numpy as np

N, H, W, C = 8, 48, 48, 128
HW = H * W            # 2304
NT = HW // 128        # 18
EPS = 1e-5
CLIP_HI = float(np.float32(float(H - 1) - EPS))  # 46.99999 (f32)

NCHUNK = 6            # output store granularity (finer -> smaller final store)
TPC = NT // NCHUNK    # tiles per store chunk = 3

_CACHE = {}


def _build_nc():
    from contextlib import ExitStack

    import concourse.bacc as bacc
    import concourse.bass as bass
    import concourse.mybir as mybir
    import concourse.tile as tile

    dt = mybir.dt
    f32, f16, i32 = dt.float32, dt.float16, dt.int32
    Alu = mybir.AluOpType

    nc = bacc.Bacc("TRN2", target_bir_lowering=False, debug=False, num_devices=N)

    xq = nc.dram_tensor("xq", [HW, 4 * C], f16, kind="ExternalInput")
    gcoef2 = nc.dram_tensor("gcoef2", [128, 4], f32, kind="ExternalInput")
    gcoef = nc.dram_tensor("gcoef", [128, 2 * NT], f32, kind="ExternalInput")
    gall = nc.dram_tensor("gall", [128, 16 * NT], f32, kind="ExternalInput")
    outp = nc.dram_tensor("outp", [128, HW], f16, kind="ExternalOutput")

    with tile.TileContext(nc) as tc, ExitStack() as ctx:
        pool = ctx.enter_context(tc.tile_pool(name="p", bufs=1))
        v = nc.vector
        gp = nc.gpsimd

        # ---- load grid layouts (HWDGE); gcoef2 first (idx critical path) ---
        g_c2 = pool.tile([128, 4], f32)
        g_coef = pool.tile([128, 2 * NT], f32)
        g_all = pool.tile([128, 16 * NT], f32)
        nc.sync.dma_start(g_c2[:], gcoef2.ap())
        nc.sync.dma_start(g_coef[:], gcoef.ap())
        nc.sync.dma_start(g_all[:], gall.ap())

        # ---- DVE idx chain: cs = clip(g)-0.5 (2 TS), int cast (round-to-
        # nearest == floor for non-integer coords; exactly-integer coords may
        # floor one lower, giving the identical bilinear result as the weight
        # crosses 0/1), then lin = r*48+s in int32 directly. Tiles 0-1 first
        # from the tiny gcoef2 (its own first-in-kernel DMA): the launch
        # train - which paces the whole pipeline - starts ~1.2us earlier.
        cs0 = pool.tile([128, 4], f32)
        fi0 = pool.tile([128, 4], i32)
        idx0 = pool.tile([128, 2], i32)
        gts = [None] * NT
        with tc.high_priority():
            v.tensor_scalar(cs0[:], g_c2[:], EPS, CLIP_HI, Alu.max, Alu.min)
            v.tensor_scalar(cs0[:], cs0[:], -0.5, None, Alu.add)
            v.tensor_copy(fi0[:], cs0[:])
            v.scalar_tensor_tensor(
                idx0[:], fi0[:, 0::2], W, fi0[:, 1::2], Alu.mult, Alu.add
            )
            for t in (0, 1):
                gt_t = pool.tile([128, 4 * C], f16, tag=f"G{t}", name=f"g{t}")
                gp.indirect_dma_start(
                    out=gt_t[:],
                    out_offset=None,
                    in_=xq.ap(),
                    in_offset=bass.IndirectOffsetOnAxis(
                        ap=idx0[:, t: t + 1], axis=0
                    ),
                )
                gts[t] = gt_t

        # remaining 16 tiles' idx from the full gcoef, then their launches
        cs = pool.tile([128, 2 * NT], f32)
        fi = pool.tile([128, 2 * NT], i32)
        idx = pool.tile([128, NT], i32)
        v.tensor_scalar(cs[:, 4:], g_coef[:, 4:], EPS, CLIP_HI, Alu.max,
                        Alu.min)
        v.tensor_scalar(cs[:, 4:], cs[:, 4:], -0.5, None, Alu.add)
        v.tensor_copy(fi[:, 4:], cs[:, 4:])
        v.scalar_tensor_tensor(
            idx[:, 2:], fi[:, 4::2], W, fi[:, 5::2], Alu.mult, Alu.add
        )
        for t in range(2, NT):
            gt_t = pool.tile([128, 4 * C], f16, tag=f"G{t}", name=f"g{t}")
            gp.indirect_dma_start(
                out=gt_t[:],
                out_offset=None,
                in_=xq.ap(),
                in_offset=bass.IndirectOffsetOnAxis(ap=idx[:, t: t + 1],
                                                    axis=0),
            )
            gts[t] = gt_t
        # fold tiles 0-1 working values into the full tiles for coefficients
        v.tensor_copy(cs[:, 0:4], cs0[:])
        v.tensor_copy(fi[:, 0:4], fi0[:])

        # ---- DVE: disk mask = AND over batches/coords of in-bounds test ----
        g_all3 = g_all[:].rearrange("p (t m) -> p t m", m=16)
        mn = pool.tile([128, NT], f32)
        mx = pool.tile([128, NT], f32)
        v.tensor_reduce(mn[:], g_all3, mybir.AxisListType.X, Alu.min)
        v.tensor_reduce(mx[:], g_all3, mybir.AxisListType.X, Alu.max)
        mge = pool.tile([128, NT], f32)
        mle = pool.tile([128, NT], f32)
        v.tensor_scalar(mge[:], mn[:], -0.5, None, Alu.is_ge)
        v.tensor_scalar(mle[:], mx[:], float(H) - 0.5, None, Alu.is_le)
        maskf = pool.tile([128, NT], f32)
        v.tensor_tensor(maskf[:], mge[:], mle[:], Alu.mult)

        # ---- DVE: bilinear coefficients (f32, [128, NT]) -------------------
        # fr = frac - 0.5 (shifted); the +0.5 folds into the STT/TS ops.
        flr = pool.tile([128, 2 * NT], f32)
        v.tensor_copy(flr[:], fi[:])
        fr = pool.tile([128, 2 * NT], f32)
        v.tensor_tensor(fr[:], cs[:], flr[:], Alu.subtract)
        fra, frb = fr[:, 0::2], fr[:, 1::2]
        fa0 = pool.tile([128, NT], f32)      # 1-a = 0.5 - fra
        fb0 = pool.tile([128, NT], f32)      # 1-b = 0.5 - frb
        v.tensor_scalar(fa0[:], fra, -1.0, 0.5, Alu.mult, Alu.add)
        v.tensor_scalar(fb0[:], frb, -1.0, 0.5, Alu.mult, Alu.add)
        fa0m = pool.tile([128, NT], f32)     # (1-a)*mask
        fa1m = pool.tile([128, NT], f32)     # a*mask = (fra+0.5)*mask
        v.tensor_tensor(fa0m[:], fa0[:], maskf[:], Alu.mult)
        v.scalar_tensor_tensor(fa1m[:], fra, 0.5, maskf[:], Alu.add, Alu.mult)
        c00 = pool.tile([128, NT], f32)
        c01 = pool.tile([128, NT], f32)
        c10 = pool.tile([128, NT], f32)
        c11 = pool.tile([128, NT], f32)
        v.tensor_tensor(c00[:], fa0m[:], fb0[:], Alu.mult)
        v.scalar_tensor_tensor(c01[:], frb, 0.5, fa0m[:], Alu.add, Alu.mult)
        v.tensor_tensor(c10[:], fa1m[:], fb0[:], Alu.mult)
        v.scalar_tensor_tensor(c11[:], frb, 0.5, fa1m[:], Alu.add, Alu.mult)

        # ---- combine: per tile out = c00*A + c01*B + c10*C + c11*D ---------
        # ACT does two products (per-partition activation scale); DVE does
        # two fused multiply-adds + the final add. All data fp16.
        outs = []
        for k in range(NCHUNK):
            out_k = pool.tile([128, TPC * C], f16, tag=f"O{k}", name=f"o{k}")
            outs.append(out_k)
            for tt in range(TPC):
                t = k * TPC + tt
                g = gts[t]
                A = g[:, 0 * C: 1 * C]
                B = g[:, 1 * C: 2 * C]
                Cr = g[:, 2 * C: 3 * C]
                D = g[:, 3 * C: 4 * C]
                u0 = pool.tile([128, C], f16, tag=f"u0_{t}", name=f"u0_{t}")
                u1 = pool.tile([128, C], f16, tag=f"u1_{t}", name=f"u1_{t}")
                v0 = pool.tile([128, C], f16, tag=f"v0_{t}", name=f"v0_{t}")
                v1 = pool.tile([128, C], f16, tag=f"v1_{t}", name=f"v1_{t}")
                if t < 4:
                    # first tiles' products on DVE (idle at early data
                    # arrival): trims ACT's queue so a late ACT start can
                    # never backlog into the final tiles (tail variance)
                    v.tensor_scalar(u0[:], A, c00[:, t: t + 1], None,
                                    Alu.mult)
                    v.tensor_scalar(u1[:], B, c01[:, t: t + 1], None,
                                    Alu.mult)
                else:
                    nc.scalar.activation(
                        u0[:], A, mybir.ActivationFunctionType.Copy,
                        scale=c00[:, t: t + 1],
                    )
                    nc.scalar.activation(
                        u1[:], B, mybir.ActivationFunctionType.Copy,
                        scale=c01[:, t: t + 1],
                    )
                v.scalar_tensor_tensor(
                    v0[:], Cr, c10[:, t: t + 1], u0[:], Alu.mult, Alu.add
                )
                v.scalar_tensor_tensor(
                    v1[:], D, c11[:, t: t + 1], u1[:], Alu.mult, Alu.add
                )
                v.tensor_tensor(
                    out_k[:, tt * C: (tt + 1) * C], v0[:], v1[:], Alu.add
                )
            nc.sync.dma_start(
                outp.ap()[:, k * TPC * C: (k + 1) * TPC * C], out_k[:]
            )

    nc.compile()
    return nc


def _get_nc():
    if "nc" not in _CACHE:
        _CACHE["nc"] = _build_nc()
    return _CACHE["nc"]


def _stage_inputs(x, grid):
    """Build the per-core input maps (data movement / dtype cast only)."""
    x = np.ascontiguousarray(x, dtype=np.float32)
    grid = np.ascontiguousarray(grid, dtype=np.float32)
    xr = x.reshape(N, C, HW)
    gr = grid.reshape(N, HW, 2)

    # quad-row table: xq[n][k] = [xT[k], xT[k+1], xT[k+48], xT[k+49]]  (fp16)
    xt = np.zeros((N, HW + W + 2, C), dtype=np.float16)
    xt[:, :HW] = xr.transpose(0, 2, 1)
    xq = np.empty((N, HW, 4 * C), dtype=np.float16)
    xq[:, :, 0 * C: 1 * C] = xt[:, 0: HW]
    xq[:, :, 1 * C: 2 * C] = xt[:, 1: HW + 1]
    xq[:, :, 2 * C: 3 * C] = xt[:, W: HW + W]
    xq[:, :, 3 * C: 4 * C] = xt[:, W + 1: HW + W + 1]

    # gcoef[n][p, 2t+c] = gr[n, t*128+p, c]
    gc = gr.reshape(N, NT, 128, 2).transpose(0, 2, 1, 3)  # [n, p, t, c]
    gcoef = np.ascontiguousarray(gc.reshape(N, 128, 2 * NT))
    gcoef2 = np.ascontiguousarray(gcoef[:, :, 0:4])

    # gall[p, 16t+2m+c] = gr[m, t*128+p, c]   (same for all cores)
    ga = gr.reshape(N, NT, 128, 2).transpose(2, 1, 0, 3)  # [p, t, m, c]
    gall = np.ascontiguousarray(ga.reshape(128, 16 * NT))

    return [
        {"xq": xq[n], "gcoef2": gcoef2[n], "gcoef": gcoef[n], "gall": gall}
        for n in range(N)
    ]


def _unstage_output(results):
    """results[n]["outp"] is (128, 2304) f16 = [p, t*128+c] -> (N, C, H, W)."""
    out = np.empty((N, C, H, W), dtype=np.float32)
    for n in range(N):
        o = results[n]["outp"].astype(np.float32).reshape(128, NT, C)
        out[n] = o.transpose(2, 1, 0).reshape(C, H, W)   # [c, q=t*128+p]
    return out


def kernel(x, grid):
    from concourse import bass_utils

    nc = _get_nc()
    in_maps = _stage_inputs(x, grid)
    res = bass_utils.run_bass_kernel_spmd(nc, in_maps, core_ids=list(range(N)))
    return _unstage_output(res.results)
